# revision 25
# baseline (speedup 1.0000x reference)
"""Trainium2 Bass kernel for CausalSelfAttention (B=4, T=2048, C=1024, H=16)
with additive prev-prob key bias.

Sharding: 8 cores = data-parallel over B (4) x tensor-parallel over head
halves (2).  Each core computes qkv for its 8 heads, causal attention, and a
partial output projection (row-parallel W_proj); host sums the two partials
per batch at unshard time.

Per-core device algorithm (gap-free tensor queue, decoupled engines):
  - QKV/proj matmuls run bf16 (host-cast inputs; FWL weight loads, halved
    DMA); QK runs bf16, PV runs fp32r (se produced by ACT at full fp32r exp
    rate).  PSUM accumulation is always fp32; scores enter exp at fp32.
  - K^T and Q^T feature-major with head pairs stacked in the 128 partitions;
    QK^T is a row-tiled concurrent pair (K=64 at base partitions 0/64).
    Two ki are grouped per superslot so the second pair's LDWEIGHTS hides
    inside the first and the row-tiled<->full-array mode switch is paid
    once per 2 ki; PV trails by 4 ki.
  - Scores transposed (keys on partitions); softmax denominator comes from an
    EA column appended to V (M=65 PV matmuls), EA[k] = (p[k]+1e-10)**-EPS.
  - Causality: block-trimmed matmul widths; the diagonal 128x128 mask is a
    PSUM seed (identity-matmul writes -1e30 above the diagonal, QK then
    accumulates start=False) so the QK->exp->PV chain crosses only
    tensor/scalar.  No mask work on DVE.
  - The scalar (ACT) engine runs a pure exp stream; QKV-gen and projection
    generators roll across chunk boundaries as tensor-slack fillers
    (deficit-driven pull with per-pair qt/kt and per-PV v2 guards).
  - Per-(head,query) 1/denominator is SBUF-local: one SBUF->SBUF DMA moves
    the den row to partition 0, reciprocal_approx_fast on DVE, gpsimd
    partition_broadcast, then the normalize multiplies — the whole chain is
    deferred into the next pair's slots (~one item per slot) so no in-order
    queue ever waits on a round trip.  The last chunk's final projection
    outputs are held back to cover the tail den chain.
"""

import math
from contextlib import ExitStack

import numpy as np

import concourse.bass as bass
import concourse.tile as tile
from concourse import bacc, mybir

F32 = mybir.dt.float32
F32R = mybir.dt.float32r
BF16 = mybir.dt.bfloat16

B, T, C, H = 4, 2048, 1024, 16
HD = C // H          # 64
NCORES = 8
HPC = H // 2         # 8 heads per core
FPC = HPC * HD       # 512 features per core
NKT = T // 128       # 16 key tiles
NQC = T // 512       # 4 query chunks (also the x t-chunks)
NCT = C // 128       # 8 contraction tiles
EPS_BIAS = 0.1
SCALE = 1.0 / math.sqrt(HD)
NEG = -1.0e30


def build(tc, out_ap, xT, wqkv, wproj, ea, trineg_dram, id_dram):
    """Emit the per-core kernel into TileContext tc.

    out_ap : (T, C)    partial projection output (needs pair-sum on host)
    xT     : (C, T)    x[b] transposed (bf16)
    wqkv   : (C, 3*FPC) [Wq_g | Wk_g | Wv_g] columns for this head group (bf16)
    wproj  : (FPC, C)  W_proj rows for this head group (bf16)
    ea     : (T,)      (prev_probs[b] + 1e-10) ** (-EPS_BIAS)
    trineg_dram: (128,128) 0 on/below diagonal (k<=q), -1e30 above
    id_dram: (128,128) identity
    """
    nc = tc.nc
    ctx = tc.ctx
    Exp = mybir.ActivationFunctionType.Exp

    const = ctx.enter_context(tc.tile_pool(name="const", bufs=1))
    xs_pool = ctx.enter_context(tc.tile_pool(name="xs", bufs=12))
    qt_pool = ctx.enter_context(tc.tile_pool(name="qt", bufs=6))
    se_pool = ctx.enter_context(tc.tile_pool(name="se", bufs=6))
    tmp_pool = ctx.enter_context(tc.tile_pool(name="tmp", bufs=4))
    rec_pool = ctx.enter_context(tc.tile_pool(name="rec", bufs=2))
    scale_pool = ctx.enter_context(tc.tile_pool(name="scale", bufs=3))
    stack_pool = ctx.enter_context(tc.tile_pool(name="stack", bufs=12))
    pout_pool = ctx.enter_context(tc.tile_pool(name="pout", bufs=3))

    ps_pool = ctx.enter_context(tc.tile_pool(name="ps", bufs=2, space="PSUM"))
    st_pool = ctx.enter_context(tc.tile_pool(name="st", bufs=2, space="PSUM"))
    y_pool = ctx.enter_context(tc.tile_pool(name="y", bufs=2, space="PSUM"))

    # ---- constants / persistent buffers ----
    # DMA emission order is sync-queue order: first weight c-tile and the
    # chunk-0 x tiles go first so the first Q matmul can start ~4us in;
    # the rest of the weights and wp (only needed by proj, ~80us in) follow.
    qts_store = {}
    kts_store = {}
    v2_done = {}
    stacks_store = {}
    xs_store = {}

    wq_sb = const.tile([128, NCT, 3 * FPC], BF16, name="wq_sb")     # 24KB/p
    wqkv3 = wqkv.rearrange("(c p) f -> p c f", p=128)
    nc.sync.dma_start(out=wq_sb[:, 0, :], in_=wqkv3[:, 0, :])

    def load_xs(qc):
        xs_tiles = []
        for c in range(NCT):
            xs = xs_pool.tile([128, 512], BF16, tag="xs", name=f"xs_{qc}_{c}")
            nc.sync.dma_start(
                out=xs, in_=xT[c * 128:(c + 1) * 128, qc * 512:(qc + 1) * 512]
            )
            xs_tiles.append(xs)
        xs_store[qc] = xs_tiles

    load_xs(0)

    kt = const.tile([128, HPC // 2, T], BF16, name="kt")            # 16KB/p
    v2 = const.tile([128, NKT, HPC, HD + 1], F32R, name="v2")       # 33.3KB/p
    eacol = const.tile([128, NKT], F32, name="eacol")
    nc.sync.dma_start(out=eacol, in_=ea.rearrange("(k p) -> p k", p=128))
    tn_f32 = const.tile([128, 128], F32, name="tn_f32")
    nc.sync.dma_start(out=tn_f32, in_=trineg_dram[:, :])
    id_f32 = const.tile([128, 128], F32, name="id_f32")
    nc.sync.dma_start(out=id_f32, in_=id_dram[:, :])
    for c in range(1, NCT):
        nc.sync.dma_start(out=wq_sb[:, c, :], in_=wqkv3[:, c, :])

    wp_sb = const.tile([128, FPC // 128, C], BF16, name="wp_sb")    # 8KB/p
    wproj3 = wproj.rearrange("(i p) c -> p i c", p=128)
    for i in range(FPC // 128):
        nc.sync.dma_start(out=wp_sb[:, i, :], in_=wproj3[:, i, :])

    ones8 = const.tile([128, HPC], F32, name="ones8")
    nc.vector.memset(ones8, 1.0)
    trineg = const.tile([128, 128], BF16, name="trineg")
    nc.vector.tensor_copy(trineg, tn_f32)
    id128 = const.tile([128, 128], BF16, name="id128")
    nc.vector.tensor_copy(id128, id_f32)

    # EA columns of v2 (column HD of each head's slot)
    for kt_i in range(NKT):
        nc.vector.tensor_scalar(
            out=v2[:, kt_i, :, HD:HD + 1],
            in0=ones8.unsqueeze(2),
            scalar1=eacol[:, kt_i:kt_i + 1],
            scalar2=None,
            op0=mybir.AluOpType.mult,
        )

    def qk_item(qc, p):
        """Q^T then K^T for head pair p of chunk qc."""
        xs_tiles = xs_store[qc]
        ps = ps_pool.tile([128, 512], F32, tag="ps", name=f"qps_{qc}_{p}")
        for c in range(NCT):
            nc.tensor.matmul(
                ps,
                wq_sb[:, c, p * 128:(p + 1) * 128],
                xs_tiles[c],
                start=(c == 0),
                stop=(c == NCT - 1),
                skip_group_check=True,
            )
            if c == 3:
                yield 900
        qt = qt_pool.tile([128, 512], BF16, tag="qt", name=f"qt_{qc}_{p}")
        nc.vector.tensor_copy(qt, ps)
        qts_store.setdefault(qc, {})[p] = qt
        yield 950
        ps = ps_pool.tile([128, 512], F32, tag="ps", name=f"kps_{qc}_{p}")
        for c in range(NCT):
            nc.tensor.matmul(
                ps,
                wq_sb[:, c, FPC + p * 128:FPC + (p + 1) * 128],
                xs_tiles[c],
                start=(c == 0),
                stop=(c == NCT - 1),
                skip_group_check=True,
            )
            if c == 3:
                yield 900
        nc.vector.tensor_copy(kt[:, p, qc * 512:(qc + 1) * 512], ps)
        kts_store.setdefault(qc, {})[p] = True
        yield 950

    def v_item(qc, j):
        xs_tiles = xs_store[qc]
        kt_i = qc * 4 + j
        ps = ps_pool.tile([128, 512], F32, tag="ps", name=f"vps_{qc}_{j}")
        for c in range(NCT):
            nc.tensor.matmul(
                ps,
                xs_tiles[c][:, j * 128:(j + 1) * 128],
                wq_sb[:, c, 2 * FPC:3 * FPC],
                start=(c == 0),
                stop=(c == NCT - 1),
                skip_group_check=True,
            )
            if c == 3:
                yield 900
        nc.vector.tensor_scalar(
            out=v2[:, kt_i, :, 0:HD],
            in0=ps.rearrange("p (h d) -> p h d", h=HPC),
            scalar1=eacol[:, kt_i:kt_i + 1],
            scalar2=None,
            op0=mybir.AluOpType.mult,
        )
        v2_done[(qc, j)] = True
        yield 950

    def gen_chunk(qc):
        """One t-chunk's pre-attention work as resumable items, each yielding
        its approximate tensor-engine ns."""
        load_xs(qc)
        yield 0
        order = [qk_item(qc, 0), v_item(qc, 0), v_item(qc, 1),
                 qk_item(qc, 1), v_item(qc, 2), v_item(qc, 3),
                 qk_item(qc, 2), qk_item(qc, 3)]
        for it in order:
            yield from it

    def den_chain(qc, p, tmp):
        """Deferred denominator chain for pair (qc,p), all SBUF-local: move
        the den row to partition 0 (one SBUF->SBUF DMA), approx-reciprocal it
        (DVE), gpsimd partition_broadcast, then the normalize multiplies.
        Pulled ~one item per slot during the NEXT pair so every item's
        dependency is already satisfied when its engine reaches it."""
        dr = rec_pool.tile([1, 1024], F32, tag="dr", name=f"dr_{qc}_{p}")
        nc.sync.dma_start(out=dr[0:1, :], in_=tmp[HD:HD + 1, :])
        yield 0
        yield 0
        rr = rec_pool.tile([1, 1024], F32, tag="rr", name=f"rr_{qc}_{p}")
        nc.vector.reciprocal_approx_fast(out=rr[0:1, :], in_=dr[0:1, :])
        yield 0
        bc = scale_pool.tile([64, 1024], F32, tag="bc", name=f"bc_{qc}_{p}")
        nc.gpsimd.partition_broadcast(bc[0:64, :], rr[0:1, :])
        yield 0
        yield 0
        stack = stack_pool.tile([128, 512], BF16, tag="stack",
                                name=f"stk_{qc}_{p}")
        nc.vector.tensor_mul(stack[0:64, :], tmp[0:64, 0:512], bc[0:64, 0:512])
        yield 0
        stkB = tmp_pool.tile([64, 512], BF16, tag="stkB", bufs=4,
                             name=f"skB_{qc}_{p}")
        nc.vector.tensor_mul(stkB[0:64, :], tmp[0:64, 512:1024], bc[0:64, 512:1024])
        nc.sync.dma_start(out=stack[64:128, :], in_=stkB[0:64, :])
        stacks_store.setdefault(qc, {})[p] = stack
        yield 0

    tail_hold = []

    def gen_proj(qc):
        stacks = stacks_store[qc]
        pool = y_pool if qc == NQC - 1 else ps_pool
        tag = "y" if qc == NQC - 1 else "ps"

        def emit(tq, ch):
            row0 = qc * 512 + tq * 128
            ps = pool.tile([128, 512], F32, tag=tag,
                           name=f"pps_{qc}_{tq}_{ch}")
            for p in range(HPC // 2):
                nc.tensor.matmul(
                    ps,
                    stacks[p][:, tq * 128:(tq + 1) * 128],
                    wp_sb[:, p, ch * 512:(ch + 1) * 512],
                    start=(p == 0),
                    stop=(p == HPC // 2 - 1),
                    skip_group_check=True,
                )
            pout = pout_pool.tile([128, 512], F32, tag="pout",
                                  name=f"po_{qc}_{tq}_{ch}")
            nc.vector.tensor_copy(pout, ps)
            nc.sync.dma_start(
                out=out_ap[row0:row0 + 128, ch * 512:(ch + 1) * 512],
                in_=pout,
            )

        items = [(tq, ch) for tq in range(4) for ch in range(2)]
        head_items = items[:4] if qc == NQC - 2 else items

        def rest():
            for tq, ch in items[len(head_items):]:
                emit(tq, ch)
                yield 1100

        for tq, ch in head_items:
            emit(tq, ch)
            yield 1100
        if len(head_items) < len(items):
            tail_hold.append(rest())

    # chunk 0: pair-0 Q/K and all V upfront; remaining Q/K interleave
    # with chunk-0 attention as fillers (per-pair guard below)
    for it in (qk_item(0, 0), v_item(0, 0), v_item(0, 1),
               v_item(0, 2), v_item(0, 3)):
        for _ in it:
            pass

    def gen_chunk0_rest():
        for it in (qk_item(0, 1), qk_item(0, 2), qk_item(0, 3)):
            yield from it

    deferred = []      # den chains: paced at most one item per pull call
    mains = [gen_chunk0_rest()]   # gen / proj generators: budget-filled
    proj_pending = []  # qc values whose proj hasn't been queued yet

    def pull_ns(budget):
        # 1. at most one deferred (den-chain) item per call
        if deferred:
            try:
                budget -= next(deferred[0]) or 0
            except StopIteration:
                deferred.pop(0)
        # 2. unlock proj(qc') once its stacks are all written
        if proj_pending and len(stacks_store.get(proj_pending[0], {})) == 4:
            mains.append(gen_proj(proj_pending.pop(0)))
        # 3. budget-fill from main generators, in order
        while budget > 0 and mains:
            try:
                est = next(mains[0])
                budget -= est if est is not None else 600
            except StopIteration:
                mains.pop(0)
                if (proj_pending
                        and len(stacks_store.get(proj_pending[0], {})) == 4):
                    mains.append(gen_proj(proj_pending.pop(0)))

    for qc in range(NQC):
        if qc + 1 < NQC:
            mains.append(gen_chunk(qc + 1))
        if qc > 0:
            proj_pending.append(qc - 1)

        nki = 4 * qc + 4
        for p in range(HPC // 2):
            while (qts_store.get(qc, {}).get(p) is None
                   or kts_store.get(qc, {}).get(p) is None):
                pull_ns(2000)
            qt = qts_store[qc][p]
            yA = y_pool.tile([128, 512], F32, tag="y", name=f"yA_{qc}_{p}")
            yB = y_pool.tile([128, 512], F32, tag="y", name=f"yB_{qc}_{p}")

            def n0_of(k):
                r = k - 4 * qc
                return 128 * r if r > 0 else 0

            ses = {}

            def emit_pv(k):
                if k >= 4 * qc:
                    while not v2_done.get((qc, k - 4 * qc)):
                        pull_ns(1500)
                n0p = n0_of(k)
                se3p = ses.pop(k)
                nc.tensor.matmul(
                    yA[0:HD + 1, n0p:512], v2[:, k, 2 * p, :],
                    se3p[:, 0, n0p:512],
                    start=(k == 0), stop=(k == nki - 1), skip_group_check=True,
                )
                nc.tensor.matmul(
                    yB[0:HD + 1, n0p:512], v2[:, k, 2 * p + 1, :],
                    se3p[:, 1, n0p:512],
                    start=(k == 0), stop=(k == nki - 1), skip_group_check=True,
                )
                return 2 * ((512 - n0p) / 2.4 + 20)

            # superslots: two ki per slot — QK pairs run back-to-back so
            # the second pair's LDWEIGHTS hides inside the first and the
            # row-tiled<->full-array mode switch is paid once per 2 ki.
            # PV trails by 4 ki (2 superslots).
            for s in range(nki // 2):
                k0, k1 = 2 * s, 2 * s + 1
                tensor_ns = 0.0
                if s >= 2:
                    tensor_ns += emit_pv(2 * s - 4)
                    tensor_ns += emit_pv(2 * s - 3)
                st3s = {}
                exp_ns = 0.0
                for k in (k0, k1):
                    n0 = n0_of(k)
                    st = st_pool.tile([128, 1024], F32, tag="st",
                                      name=f"st_{qc}_{p}_{k}")
                    st3s[k] = st.rearrange("p (h q) -> p h q", h=2)
                    if k - 4 * qc >= 0:
                        # causal mask: seed the diagonal block with -1e30
                        # above the diagonal; QK accumulates on top
                        nc.tensor.matmul(
                            st3s[k][:, 0, n0:n0 + 128], id128, trineg,
                            start=True, stop=False, skip_group_check=True,
                        )
                        nc.tensor.matmul(
                            st3s[k][:, 1, n0:n0 + 128], id128, trineg,
                            start=True, stop=False, skip_group_check=True,
                        )
                        tensor_ns += 115
                for k in (k0, k1):
                    n0 = n0_of(k)
                    diag = k - 4 * qc >= 0
                    st3 = st3s[k]
                    kslice = slice(k * 128, (k + 1) * 128)
                    nc.tensor.matmul(
                        st3[:, 0, n0:512], kt[0:64, p, kslice],
                        qt[0:64, n0:512],
                        start=not diag, stop=True, skip_group_check=True,
                    )
                    nc.tensor.matmul(
                        st3[:, 1, n0:512], kt[64:128, p, kslice],
                        qt[64:128, n0:512],
                        start=not diag, stop=True, skip_group_check=True,
                    )
                    se = se_pool.tile([128, 1024], F32R, tag="se",
                                      name=f"se_{qc}_{p}_{k}")
                    se3 = se.rearrange("p (h q) -> p h q", h=2)
                    nc.scalar.activation(
                        se3[:, :, n0:512], st3[:, :, n0:512], Exp, scale=SCALE
                    )
                    ses[k] = se3
                    tensor_ns += (512 - n0) / 2.4 + 25
                    exp_ns += (2 * (512 - n0) + 352) / 1.2
                pull_ns(exp_ns - tensor_ns)

            emit_pv(nki - 4)
            emit_pv(nki - 3)
            pull_ns(900)
            emit_pv(nki - 2)
            emit_pv(nki - 1)

            # evacuate y^T + denominator rows (DVE; scalar stays pure exp)
            tmp = tmp_pool.tile([128, 1024], F32, tag="tmp", name=f"tmp_{qc}_{p}")
            nc.vector.tensor_copy(tmp[0:HD + 1, 0:512], yA[0:HD + 1, :])
            nc.vector.tensor_copy(tmp[0:HD + 1, 512:1024], yB[0:HD + 1, :])
            deferred.append(den_chain(qc, p, tmp))
            pull_ns(1200)


    # tail: drain leftover fillers; reserved proj work covers the final
    # den-chain latency
    while mains:
        pull_ns(10000)
    while deferred or tail_hold:
        if deferred:
            pull_ns(0)
        if tail_hold:
            try:
                next(tail_hold[0])
            except StopIteration:
                tail_hold.pop(0)
    while proj_pending:
        qcq = proj_pending.pop(0)
        for _ in gen_proj(qcq):
            pass
    for _ in gen_proj(NQC - 1):
        pass


def make_nc():
    nc = bacc.Bacc("TRN2", target_bir_lowering=False, debug=False,
                   num_devices=NCORES)
    xT = nc.dram_tensor("xT", [C, T], BF16, kind="ExternalInput")
    wqkv = nc.dram_tensor("wqkv", [C, 3 * FPC], BF16, kind="ExternalInput")
    wproj = nc.dram_tensor("wproj", [FPC, C], BF16, kind="ExternalInput")
    ea = nc.dram_tensor("ea", [T], F32, kind="ExternalInput")
    out = nc.dram_tensor("out", [T, C], F32, kind="ExternalOutput")
    kq = np.arange(128)
    trineg_np = np.where(kq[:, None] <= kq[None, :], 0.0, NEG).astype(np.float32)
    trineg_dram = nc.inline_tensor(trineg_np, name="trineg_const")
    id_np = np.eye(128, dtype=np.float32)
    id_dram = nc.inline_tensor(id_np, name="id_const")
    with ExitStack() as ctx:
        tc = ctx.enter_context(tile.TileContext(nc))
        tc.ctx = ctx
        build(tc, out[:, :], xT[:, :], wqkv[:, :], wproj[:, :], ea[:],
              trineg_dram, id_dram)
    nc.compile()
    return nc


def shard_inputs(x, prev_probs, W_attn, W_proj):
    import ml_dtypes

    bf16 = ml_dtypes.bfloat16
    in_maps = []
    for core in range(NCORES):
        b, g = divmod(core, 2)
        xT = np.ascontiguousarray(x[b].T)
        wq = W_attn[:, g * FPC:(g + 1) * FPC]
        wk = W_attn[:, C + g * FPC:C + (g + 1) * FPC]
        wv = W_attn[:, 2 * C + g * FPC:2 * C + (g + 1) * FPC]
        wqkv = np.ascontiguousarray(np.concatenate([wq, wk, wv], axis=1))
        wproj = np.ascontiguousarray(W_proj[g * FPC:(g + 1) * FPC, :])
        ea = np.power(prev_probs[b] + np.float32(1e-10), np.float32(-EPS_BIAS))
        in_maps.append(
            {
                "xT": xT.astype(bf16),
                "wqkv": wqkv.astype(bf16),
                "wproj": wproj.astype(bf16),
                "ea": ea.astype(np.float32),
            }
        )
    return in_maps


_CACHED_NC = None


def kernel(x, prev_probs, W_attn, W_proj, trace=False, tmpdir=None):
    global _CACHED_NC
    from concourse.bass_utils import run_bass_kernel_spmd

    x = np.asarray(x, dtype=np.float32)
    prev_probs = np.asarray(prev_probs, dtype=np.float32)
    W_attn = np.asarray(W_attn, dtype=np.float32)
    W_proj = np.asarray(W_proj, dtype=np.float32)

    if _CACHED_NC is None:
        _CACHED_NC = make_nc()
    nc = _CACHED_NC

    in_maps = shard_inputs(x, prev_probs, W_attn, W_proj)
    res = run_bass_kernel_spmd(
        nc, in_maps, core_ids=list(range(NCORES)), trace=trace, tmpdir=tmpdir
    )
    parts = [r["out"] for r in res.results]
    out = np.empty((B, T, C), dtype=np.float32)
    for b in range(B):
        out[b] = parts[2 * b] + parts[2 * b + 1]
    kernel.last_results = res
    return out
